# revision 1
# baseline (speedup 1.0000x reference)
"""Bass/Trainium2 kernel for nn_Attn (32,4096,512 attention pooling).

  energy = tanh(x @ W.T); ae = energy @ v; w = softmax(ae, axis=T)
  out[b] = sum_t w[b,t] * x[b,t,:]

Strategy (8 NeuronCores, data-parallel over B, 4 batches/core):
  - The energy matmul runs in fp8e4m3 with MatmulPerfMode.DoubleRow
    (K=256/instruction, 0.5 cyc/col): x8^T (host-pre-transposed fp8 copy,
    8.4MB/core) against W8 = fp8(16*W) (the x16 avoids fp8 subnormals on
    the small W entries; undone for free via the tanh activation's
    scale=1/16).
  - fp8 W quantization error is cancelled to first order by a rank-1
    correction ae += x8 . r, r = (v*E[tanh']) @ (W - W8/16), computed on
    host from W alone and applied as two more DoubleRow matmuls into the
    same PSUM row.
  - ae row assembly: v-dot is split to balance engines. Per 512-token
    block, PSUM row pv = 64*ae accumulates: corr (fp8 DR), gc0 via
    lhsT=64*v, and gc1..3 via a DVE tensor_scalar/tensor_tensor chain
    (the only DVE ops with the 2x 16-bit mode; scalar_tensor_tensor has
    none) summed through lhsT=64.0. The x64 keeps one DVE copy (PSUM f32
    -> f16) per block; exp's scale=1/64 undoes it.
  - [1,T] ae row -> [128,NT] relayout via tiny PE transpose matmuls
    (no DRAM round-trip, no DMA-xbar transposes anywhere).
  - softmax skips the max subtraction (|ae| <= sum|v| ~ 25, exp fits
    f32); weights stay unnormalized, output scaled by 1/S at the end.
  - weighted sum stays bf16 (natural-layout x, 16.8MB/core): fp8 there
    would push rel-err past the gate.
"""

import numpy as np
import ml_dtypes
from contextlib import ExitStack

import bass_rust
import concourse.bass as bass
import concourse.mybir as mybir
import concourse.tile as tile
from concourse.bass_utils import run_bass_kernel_spmd

# ---------------------------------------------------------------------------
# Workaround: this container's walrus accepts only ONE sem-wait per
# instruction. Tile's sem-assignment can attach several. Split the extras
# onto same-engine NoOps inserted immediately before the instruction.


def _split_excess_waits(nc, max_waits=1):
    n_split = 0
    for fn in nc.m.functions:
        for blk in fn.blocks:
            new = []
            changed = False
            for inst in blk.instructions:
                si = inst.sync_info
                waits = list(si.on_wait) if si is not None else []
                if len(waits) > max_waits:
                    for w in waits[:-max_waits]:
                        nop = mybir.InstNoOp(
                            name=nc.get_next_instruction_name(),
                            engine=inst.engine,
                            ins=[],
                            outs=[],
                            sync_info=bass_rust.SyncInfo(
                                on_wait=[w], on_update=[]
                            ),
                        )
                        new.append(nop)
                        n_split += 1
                    inst.sync_info = bass_rust.SyncInfo(
                        on_wait=waits[-max_waits:], on_update=list(si.on_update)
                    )
                    changed = True
                new.append(inst)
            if changed:
                blk.instructions = new
    return n_split
# ---------------------------------------------------------------------------

B, T, H = 32, 4096, 512
N_CORES = 8
B_LOC = B // N_CORES          # batches per core
PC = 128                      # partitions
HC = H // PC                  # 4 h-chunks
GC = H // PC                  # 4 g-chunks
TBLK = 512                    # tokens per pipeline block
NBLK = T // TBLK              # 8 blocks per batch
NT = T // PC                  # 32 token subtiles per batch

WS = 16.0                     # W fp8 pre-scale
KR = 64.0                     # pv row carries 64*ae; r8 = fp8(64*r)

BF16 = mybir.dt.bfloat16
F16 = mybir.dt.float16
F32 = mybir.dt.float32
F8 = mybir.dt.float8e4
AF = mybir.ActivationFunctionType
DR = mybir.MatmulPerfMode.DoubleRow
MULT = mybir.AluOpType.mult
ADD = mybir.AluOpType.add


def _build_program(reps=1, split_waits=True):
    nc = bass.Bass()
    x_d = nc.declare_dram_parameter("x", [B_LOC, T, H], BF16, isOutput=False)
    xt_d = nc.declare_dram_parameter("xt", [B_LOC, H, T], F8, isOutput=False)
    w8_d = nc.declare_dram_parameter("w8", [H, H], F8, isOutput=False)
    r8_d = nc.declare_dram_parameter("r8", [H, 1], F8, isOutput=False)
    v64_d = nc.declare_dram_parameter("v64", [PC, GC], BF16, isOutput=False)
    v32_d = nc.declare_dram_parameter("v32", [PC, GC], F32, isOutput=False)
    out_d = nc.declare_dram_parameter("out", [B_LOC, 3, H], F32, isOutput=True)
    s_d = nc.declare_dram_parameter("s", [B_LOC, 1], F32, isOutput=True)

    with tile.TileContext(nc) as tc, ExitStack() as ctx:
        singles = ctx.enter_context(tc.tile_pool(name="singles", bufs=1))
        xnatp = ctx.enter_context(tc.tile_pool(name="xnat", bufs=3))
        xtp = ctx.enter_context(tc.tile_pool(name="xt", bufs=2))
        tanhp = ctx.enter_context(tc.tile_pool(name="tanh", bufs=4))
        dvep = ctx.enter_context(tc.tile_pool(name="dve", bufs=6))
        wfp = ctx.enter_context(tc.tile_pool(name="wf", bufs=3))
        wcp = ctx.enter_context(tc.tile_pool(name="wc", bufs=2))
        smallp = ctx.enter_context(tc.tile_pool(name="small", bufs=4))
        osbp = ctx.enter_context(tc.tile_pool(name="osb", bufs=2))
        pep = ctx.enter_context(tc.tile_pool(name="pe", bufs=2, space="PSUM"))
        pvp = ctx.enter_context(tc.tile_pool(name="pv", bufs=1, space="PSUM"))
        pwp = ctx.enter_context(tc.tile_pool(name="pw", bufs=2, space="PSUM"))
        pop = ctx.enter_context(tc.tile_pool(name="po", bufs=1, space="PSUM"))

        # constants
        w8_sb = singles.tile([PC, HC, H], F8)
        nc.sync.dma_start(out=w8_sb, in_=w8_d.rearrange("(hc p) g -> p hc g", p=PC))
        # hc stride padded to 16B: DoubleRow Ldweights requires 16B-aligned
        # outer steps on the weight AP (s3_lw_dual_fp8_restrictions)
        r8_sb = singles.tile([PC, HC, 16], F8)
        nc.sync.dma_start(out=r8_sb[:, :, 0:1],
                          in_=r8_d.rearrange("(hc p) one -> p hc one", p=PC))
        v64_sb = singles.tile([PC, GC], BF16)
        nc.sync.dma_start(out=v64_sb, in_=v64_d[:, :])
        v32_sb = singles.tile([PC, GC], F32)
        nc.sync.dma_start(out=v32_sb, in_=v32_d[:, :])
        ones64_sb = singles.tile([PC, 1], BF16)
        nc.vector.memset(ones64_sb, KR)
        ident_sb = singles.tile([1, 1], F32)
        nc.vector.memset(ident_sb, 1.0)

        pools = dict(
            xnatp=xnatp, xtp=xtp, tanhp=tanhp, dvep=dvep, wfp=wfp, wcp=wcp,
            smallp=smallp, osbp=osbp, pep=pep, pvp=pvp, pwp=pwp, pop=pop,
        )
        consts = dict(
            w8=w8_sb, r8=r8_sb, v64=v64_sb, v32=v32_sb,
            ones64=ones64_sb, ident=ident_sb,
        )

        def body():
            _pipelined_body(nc, x_d, xt_d, (out_d, s_d), pools, consts)

        if reps == 1:
            body()
        else:
            with tc.For_i(0, reps, 1):
                body()

    if split_waits:
        _split_excess_waits(nc)
    return nc


def _pipelined_body(nc, x_d, xt_d, out_d, P, C):
    """Two-level software pipeline over the 32 (batch, block) units.

    Per unit i: emit energy+tanh+DVE-chain for block i, the pv assembly
    (corr/v-dot matmuls + row copy) for block i-1, and the row->partition
    transposes for block i-2, so the in-order PE queue always has energy
    matmuls in front of chain-dependent stragglers. Batch finish (exp,
    weighted sum, output) is emitted as soon as its last transposes are."""
    S = {"xn": {}, "xt": {}, "pe": {}, "tE": {}, "pv": {}, "wf": {}, "pw": {}}

    def dmas(b):
        # xt first: the energy matmuls gate on it, and the SP queue is serial
        xt = P["xtp"].tile([PC, HC, T], F8, tag="xt")
        nc.sync.dma_start(out=xt, in_=xt_d[b].rearrange("(hc p) t -> p hc t", p=PC))
        xn = P["xnatp"].tile([PC, NT, H], BF16, tag="xn")
        xv = x_d[b].rearrange("(n p) h -> p n h", p=PC)
        # spread the 4.2MB natural-x load across the two HWDGE queues
        # (transfer time occupies the issuing queue; only needed by the
        # weighted sum, a full batch later)
        nc.sync.dma_start(out=xn[:, 0:27, :], in_=xv[:, 0:27, :])
        nc.scalar.dma_start(out=xn[:, 27:NT, :], in_=xv[:, 27:NT, :])
        S["xn"][b], S["xt"][b] = xn, xt

    def energy(b, k):
        if b not in S["xn"]:
            dmas(b)
        if k == 0:
            S["pw"][b] = P["pwp"].tile([PC, NT], F32, tag="pw", name=f"pw{b}")
        xt = S["xt"][b]
        t0 = k * TBLK
        tEs = []
        for pair in range(2):
            pe_t = P["pep"].tile([PC, 2, TBLK], F32, tag="pe")
            for w in range(2):
                gc = pair * 2 + w
                for ps in range(2):
                    nc.tensor.matmul(
                        pe_t[:, w, :],
                        lhsT=C["w8"][:, 2 * ps : 2 * ps + 2, gc * PC : (gc + 1) * PC],
                        rhs=xt[:, 2 * ps : 2 * ps + 2, t0 : t0 + TBLK],
                        start=(ps == 0),
                        stop=(ps == 1),
                        perf_mode=DR,
                    )
            tE = P["tanhp"].tile([PC, 2, TBLK], BF16, tag="tE")
            nc.scalar.activation(out=tE, in_=pe_t, func=AF.Tanh, scale=1.0 / WS)
            tEs.append(tE)
        # DVE: a123 = v1*tE1 + v2*tE2 + v3*tE3 (gc0 rides a PE v64 matmul)
        sc1 = P["dvep"].tile([PC, TBLK], BF16, tag="sc")
        sc2 = P["dvep"].tile([PC, TBLK], BF16, tag="sc")
        sc3 = P["dvep"].tile([PC, TBLK], BF16, tag="sc")
        nc.vector.tensor_scalar(out=sc1, in0=tEs[0][:, 1, :],
                                scalar1=C["v32"][:, 1:2], scalar2=None, op0=MULT)
        nc.vector.tensor_scalar(out=sc2, in0=tEs[1][:, 0, :],
                                scalar1=C["v32"][:, 2:3], scalar2=None, op0=MULT)
        nc.vector.tensor_scalar(out=sc3, in0=tEs[1][:, 1, :],
                                scalar1=C["v32"][:, 3:4], scalar2=None, op0=MULT)
        a12 = P["dvep"].tile([PC, TBLK], BF16, tag="sc")
        nc.vector.tensor_tensor(out=a12, in0=sc1, in1=sc2, op=ADD)
        a123 = P["dvep"].tile([PC, TBLK], BF16, tag="sc")
        nc.vector.tensor_tensor(out=a123, in0=a12, in1=sc3, op=ADD)
        S["tE"][(b, k)] = (tEs[0], a123)

    def pv_tail(b, k):
        tE0, a123 = S["tE"].pop((b, k))
        xt = S["xt"][b]
        t0 = k * TBLK
        pv = P["pvp"].tile([1, TBLK], F32, tag="pv")
        for ps in range(2):
            nc.tensor.matmul(
                pv,
                lhsT=C["r8"][:, 2 * ps : 2 * ps + 2, 0:1],
                rhs=xt[:, 2 * ps : 2 * ps + 2, t0 : t0 + TBLK],
                start=(ps == 0),
                stop=False,
                perf_mode=DR,
            )
        nc.tensor.matmul(pv, lhsT=C["v64"][:, 0:1], rhs=tE0[:, 0, :],
                         start=False, stop=False)
        nc.tensor.matmul(pv, lhsT=C["ones64"], rhs=a123, start=False, stop=True)
        wf = P["wfp"].tile([1, TBLK], F32, tag="wf")
        nc.vector.tensor_copy(wf, pv)
        S["wf"][(b, k)] = wf

    def transp(b, k):
        wf = S["wf"].pop((b, k))
        pw = S["pw"][b]
        for j in range(TBLK // PC):
            nc.tensor.transpose(
                pw[:, k * 4 + j : k * 4 + j + 1],
                wf[:, j * PC : (j + 1) * PC],
                C["ident"],
            )

    def finish_exp(b):
        pw = S["pw"].pop(b)
        wc = P["wcp"].tile([PC, NT], BF16, tag="wc", name=f"wc{b}")
        sexp = P["smallp"].tile([PC, 1], F32, tag="sexp", name=f"sexp{b}")
        nc.scalar.activation(out=wc, in_=pw, func=AF.Exp, scale=1.0 / KR,
                             accum_out=sexp)
        Ssum = P["smallp"].tile([1, 1], F32, tag="S", name=f"S{b}")
        nc.gpsimd.tensor_reduce(out=Ssum, in_=sexp, axis=mybir.AxisListType.C, op=ADD)
        nc.sync.dma_start(out=out_d[1][b : b + 1, :], in_=Ssum)
        S["wc"] = S.get("wc", {})
        S["wc"][b] = wc

    def finish_wsum(b):
        xn = S["xn"].pop(b)
        del S["xt"][b]
        wc = S["wc"].pop(b)
        # round-robin over FOUR PSUM rows (partitions 0/32/64/96 - legal
        # matmul output bases): consecutive accumulating matmuls into one
        # row serialize on HW; 3 rows give each chain ~2 matmuls of spacing.
        # Host adds the rows and divides by S.
        po = P["pop"].tile([65, H], F32, tag="po")
        for j in range(NT):
            r = (j % 3) * 32
            nc.tensor.matmul(
                po[r : r + 1, :],
                lhsT=wc[:, j : j + 1],
                rhs=xn[:, j, :],
                start=(j < 3),
                stop=(j >= NT - 3),
            )
        ob = P["osbp"].tile([65, H], F32, tag="ob")
        nc.vector.tensor_copy(ob, po)
        for r in range(3):
            nc.sync.dma_start(out=out_d[0][b, r : r + 1, :],
                              in_=ob[r * 32 : r * 32 + 1, :])

    units = [(b, k) for b in range(B_LOC) for k in range(NBLK)]
    NU = len(units)
    for i in range(NU + 4):
        if i < NU:
            b, k = units[i]
            energy(b, k)
            if k == 0 and b + 1 < B_LOC:
                dmas(b + 1)   # prefetch next batch early
        if 1 <= i < NU + 1:
            pv_tail(*units[i - 1])
        if 2 <= i < NU + 2:
            bb, kk = units[i - 2]
            transp(bb, kk)
            if kk == NBLK - 1:
                finish_exp(bb)
        if 4 <= i < NU + 4:
            bb, kk = units[i - 4]
            if kk == NBLK - 1:
                finish_wsum(bb)


def make_in_maps(inputs):
    """Host-side prep: dtype casts, fp8 transposed copy, rank-1 corr vector."""
    x = np.asarray(inputs["encoder_outputs"], dtype=np.float32)
    W = np.asarray(inputs["W"], dtype=np.float32)
    v = np.asarray(inputs["v"], dtype=np.float32)
    assert x.shape == (B, T, H)
    bf16 = ml_dtypes.bfloat16
    e4m3 = ml_dtypes.float8_e4m3

    xb = x.astype(bf16)
    x8t = np.ascontiguousarray(x.transpose(0, 2, 1)).astype(e4m3)  # (B,H,T)
    w8 = np.ascontiguousarray((WS * W).T).astype(e4m3)             # (H,G)
    Weff = w8.astype(np.float32).T / WS                            # (G,H)
    dW = W - Weff

    # m_g = E[tanh'(u)], u ~ N(0, sig_g^2), via Gauss-Hermite (input-indep)
    sig = np.linalg.norm(Weff, axis=1)
    gh_x, gh_w = np.polynomial.hermite_e.hermegauss(61)
    m_g = ((1.0 - np.tanh(np.outer(sig, gh_x)) ** 2) @ gh_w) / gh_w.sum()
    r = (v * m_g.astype(np.float32)) @ dW                          # (H,)
    r8 = (KR * r).astype(e4m3)[:, None]                            # (H,1)

    v64 = np.ascontiguousarray((KR * v).reshape(GC, PC).T).astype(bf16)
    v32 = np.ascontiguousarray(v.reshape(GC, PC).T).astype(np.float32)

    return [
        {
            "x": xb[c * B_LOC : (c + 1) * B_LOC],
            "xt": x8t[c * B_LOC : (c + 1) * B_LOC],
            "w8": w8,
            "r8": r8,
            "v64": v64,
            "v32": v32,
        }
        for c in range(N_CORES)
    ]


_PROGRAM = None


def _get_program():
    global _PROGRAM
    if _PROGRAM is None:
        _PROGRAM = _build_program()
    return _PROGRAM


def run(inputs, trace=False, trace_kwargs=None):
    in_maps = make_in_maps(inputs)
    nc = _get_program()
    res = run_bass_kernel_spmd(
        nc,
        in_maps,
        list(range(N_CORES)),
        trace=trace,
        **(trace_kwargs or {}),
    )
    out = np.concatenate([res.results[c]["out"] for c in range(N_CORES)], axis=0)
    s = np.concatenate([res.results[c]["s"] for c in range(N_CORES)], axis=0)
    out = out.sum(axis=1) / s
    return out.astype(np.float32), res


def kernel(**inputs):
    out, _ = run(inputs)
    return out

